# revision 28
# baseline (speedup 1.0000x reference)
"""Trainium2 Bass kernel for nn_DiffusionModel1d (batched 1-D diffusion solve).

Math: the reference solves A(K) u = f per batch row with K = exp(x) via the
Thomas algorithm, where A = G^T diag(K_hat) G, G the n x n lower-bidiagonal
difference matrix (1 on diag, -1 on subdiag) and
K_hat = (2*K_0, K_1, ..., K_{n-1}).  Hence

    u = h2 * G^{-1} diag(K_hat)^{-1} G^{-T} f
      = h2 * cumsum_j( w_j * exp(-x_j) ),   w = suffix_sum(f), w_0 halved.

Layout: TRANSPOSED (grid dim in partitions).  The prefix sum along the grid
dim becomes a per-chunk triangular matrix multiply on the Tensor engine
(lhsT[k, j] = w'_k for k <= j), which removes the Vector-engine scan that
bottlenecked the batch-major version.  Per core (1024 batch cols):

  - 16 grid chunks of 128; e = exp(-x) on ACT (fp16), waves of 2 chunks.
  - main matmul per chunk: local weighted prefix into PSUM fp32.
  - cross-chunk carries: a CAR PSUM tile accumulates SFull_c @ e_c (full
    weight columns -> the chunk total replicated on ALL partitions); one
    fp16 snapshot to SBUF per wave (ACT copy).  Because the carry is
    replicated, the carry add folds into the PSUM->SBUF evacuation as a
    DVE tensor_tensor add (engines cannot partition-broadcast, and engine
    APs must start at partition 0/32/64, so a row-per-chunk totals tile is
    not expressible).  The odd chunk of each wave gets the even chunk's
    contribution as one extra SFull matmul accumulated into its PSUM.
  - matmuls are emitted 1024 wide (2 PSUM banks) to halve instruction +
    LDWEIGHTS count on the Tensor engine.

Everything 16-bit on the wire: x as fp16, weights (triangular + full,
shipped as one [128, 4096] tensor) fp16 scaled by 2^-4 so they stay in
fp16 normal range (h2 * 2^4 applied on host), output fp16.  DMA/core =
4 MB in + 1 MB weights + 4 MB out ~ 9 MB.  Host does the transpose/
swizzle so all device DMAs are contiguous.
"""

import os
import sys

import numpy as np

sys.path.insert(0, "/opt/trn_rl_repo")

import concourse.bacc as bacc
import concourse.mybir as mybir
import concourse.tile as tile
from concourse import bass_utils

B, M = 8192, 2048
N = M - 1
NCORES = 8
BC = B // NCORES          # 1024 batch cols per core
P = 128                   # SBUF partitions
NCH = M // P              # 16 grid chunks per core
NPAIR = NCH // 2          # 8 chunk pairs (one [128, 2048] tile each)
H2 = (1.0 / N) ** 2
SW = 2.0 ** -4            # weight prescale (keeps w' in fp16 normal range)

_cached_nc = None
LAST_RESULTS = None


def _build_kernel():
    fp32 = mybir.dt.float32
    f16 = mybir.dt.float16
    nc = bacc.Bacc(
        "TRN2",
        target_bir_lowering=False,
        debug=False,
        enable_asserts=False,
        num_devices=NCORES,
    )
    x_d = nc.dram_tensor("x", (BC, 2 * BC), f16, kind="ExternalInput").ap()
    w_d = nc.dram_tensor("w", (P, 2 * M), f16, kind="ExternalInput").ap()
    o_d = nc.dram_tensor("out", (BC, 2 * BC), f16, kind="ExternalOutput").ap()

    EXP = mybir.ActivationFunctionType.Exp
    ADD = mybir.AluOpType.add

    with tile.TileContext(nc) as tc:
        with (
            tc.tile_pool(name="const", bufs=1) as cpool,
            tc.tile_pool(name="xin", bufs=NPAIR) as xpool,
            tc.tile_pool(name="ee", bufs=3) as epool,
            tc.tile_pool(name="oo", bufs=4) as opool,
            tc.tile_pool(name="cs", bufs=2) as cspool,
            tc.tile_pool(name="ps", bufs=3, space="PSUM") as pspool,
            tc.tile_pool(name="pc", bufs=1, space="PSUM") as carpool,
        ):
            # first x pair in halves so the first exp starts earlier; the
            # weight DMA goes between them on the same in-order ring.
            xts = []
            for p in range(NPAIR):
                xt = xpool.tile([P, 2 * BC], f16, tag="x")
                if p == 0:
                    # halves: the first exp starts as soon as 256 KB lands
                    nc.sync.dma_start(out=xt[:, :BC], in_=x_d[:P, :BC])
                    nc.sync.dma_start(out=xt[:, BC:], in_=x_d[:P, BC:])
                xts.append(xt)
            for p in range(1, NPAIR):
                nc.sync.dma_start(out=xts[p], in_=x_d[p * P : (p + 1) * P, :])
            # combined weights: [:, :M] = triangular, [:, M:] = full
            # columns; on the scalar HWDGE ring so the 1 MB transfer
            # streams in parallel with the x loads instead of behind them
            wt = cpool.tile([P, 2 * M], f16, tag="wt")
            nc.scalar.dma_start(out=wt, in_=w_d)
            wf = wt[:, M:]

            # HAM warmup: ~8 dummy matmuls during the startup dead time so
            # the PE clock gate is at 8/8 (2.4 GHz) when real work arrives
            warm = cpool.tile([P, 512], f16, tag="warm")
            nc.vector.memset(warm, 0.0)
            ones1 = cpool.tile([1, P], f16, tag="ones1")
            nc.vector.memset(ones1, 1.0)

            car = carpool.tile([P, BC], fp32, tag="car")
            for _ in range(8):
                nc.tensor.matmul(
                    car[:, :512],
                    lhsT=warm[:, :P],
                    rhs=warm,
                    start=True,
                    stop=True,
                    skip_group_check=True,
                )
            carsb = {}

            # exp for wave 0 (pair 0) in halves
            ets = {}
            et0 = epool.tile([P, 2 * BC], f16, tag="e")
            nc.scalar.activation(
                out=et0[:, :BC], in_=xts[0][:, :BC], func=EXP, scale=-1.0
            )
            nc.scalar.activation(
                out=et0[:, BC:], in_=xts[0][:, BC:], func=EXP, scale=-1.0
            )
            ets[0] = et0

            for p in range(NPAIR):  # wave == pair: chunks 2p, 2p+1
                if p + 1 < NPAIR:   # prefetch next wave's exp
                    et = epool.tile([P, 2 * BC], f16, tag="e")
                    nc.scalar.activation(
                        out=et, in_=xts[p + 1], func=EXP, scale=-1.0
                    )
                    ets[p + 1] = et
                ep = ets[p]
                ot = opool.tile([P, 2 * BC], f16, tag="o")
                pts = []
                for i in range(2):
                    c = 2 * p + i
                    ec = ep[:, i * BC : (i + 1) * BC]
                    pt = pspool.tile([P, BC], fp32, tag="ps")
                    pts.append(pt)
                    last = p == NPAIR - 1
                    for h in range(2):
                        hs = slice(h * 512, (h + 1) * 512)
                        nc.tensor.matmul(
                            pt[:, hs],
                            lhsT=wt[:, c * P : (c + 1) * P],
                            rhs=ec[:, hs],
                            start=True,
                            stop=(i == 0) and not last,
                        )
                    if i == 1:
                        # even chunk's full contribution into odd chunk
                        for h in range(2):
                            hs = slice(h * 512, (h + 1) * 512)
                            nc.tensor.matmul(
                                pt[:, hs],
                                lhsT=wf[:, (c - 1) * P : c * P],
                                rhs=ep[:, hs],
                                start=False,
                                stop=not last,
                            )
                    if last:
                        # inject the carry via rank-1 matmuls (TensorE is
                        # idle at the tail) so the final evacs are plain
                        # copies running on ACT and DVE in parallel
                        for h in range(2):
                            hs = slice(h * 512, (h + 1) * 512)
                            nc.tensor.matmul(
                                pt[:, hs],
                                lhsT=ones1,
                                rhs=carsb[p - 1][0:1, hs],
                                start=False,
                                stop=True,
                            )
                    # running cross-wave carry accumulator (not needed
                    # after the second-to-last wave's snapshot)
                    if p + 1 < NPAIR:
                        for h in range(2):
                            hs = slice(h * 512, (h + 1) * 512)
                            nc.tensor.matmul(
                                car[:, hs],
                                lhsT=wf[:, c * P : (c + 1) * P],
                                rhs=ec[:, hs],
                                start=(c == 0),
                                stop=(c == NCH - 3),
                                skip_group_check=True,
                            )
                # snapshot the running carry for the next wave, split
                # ~70/30 between ACT and DVE so neither engine's per-wave
                # load (ACT: exp 1.89, DVE: evac-adds 2.26) exceeds ~2.65
                if p + 1 < NPAIR:
                    cs = cspool.tile([P, BC], f16, tag="cs")
                    nc.scalar.copy(out=cs[:, :704], in_=car[:, :704])
                    nc.vector.tensor_copy(out=cs[:, 704:], in_=car[:, 704:])
                    carsb[p] = cs
                # evacuate with fused carry add (replicated on partitions);
                # wave 0 has no carry -> plain DVE copies (ACT is on exps);
                # last wave's carry is already in PSUM -> parallel copies
                for i in range(2):
                    dst = ot[:, i * BC : (i + 1) * BC]
                    if p == 0:
                        nc.vector.tensor_copy(out=dst, in_=pts[i])
                    elif p == NPAIR - 1:
                        if i == 0:
                            nc.scalar.copy(out=dst, in_=pts[i])
                        else:
                            nc.vector.tensor_copy(out=dst, in_=pts[i])
                    else:
                        nc.vector.tensor_tensor(
                            out=dst, in0=pts[i], in1=carsb[p - 1], op=ADD
                        )
                if p == NPAIR - 1:
                    # last pair: store halves as each evac would land
                    nc.sync.dma_start(
                        out=o_d[p * P : (p + 1) * P, :BC], in_=ot[:, :BC]
                    )
                    nc.sync.dma_start(
                        out=o_d[p * P : (p + 1) * P, BC:], in_=ot[:, BC:]
                    )
                else:
                    nc.sync.dma_start(out=o_d[p * P : (p + 1) * P, :], in_=ot)

    nc.compile()
    return nc


def _get_nc():
    global _cached_nc
    if _cached_nc is None:
        _cached_nc = _build_kernel()
    return _cached_nc


def _make_w(f_rhs: np.ndarray) -> np.ndarray:
    """Combined weights [128, 2M] fp16: triangular then full-column.

    W[k, 128c + j] = w'_{128c+k} * (k <= j)  (triangular, cols 0..M)
    W[k, M + 128c + j] = w'_{128c+k}         (SFull, cols M..2M)
    w' = SW * suffix_sum(f), w'_0 halved, w'_{M-1} = 0 (pad); h2/SW is
    applied on host afterwards.
    """
    w = np.cumsum(f_rhs[::-1].astype(np.float64))[::-1] * SW
    w[0] *= 0.5
    wq = np.zeros(M, np.float16)
    wq[:N] = w.astype(np.float16)
    cols = wq.reshape(NCH, P).T  # [k, c]
    mask = np.arange(P)[:, None] <= np.arange(P)[None, :]
    wtri = (cols[:, :, None] * mask[:, None, :]).reshape(P, M)
    wfull = np.broadcast_to(cols[:, :, None], (P, NCH, P)).reshape(P, M)
    return np.ascontiguousarray(
        np.concatenate([wtri, wfull], axis=1).astype(np.float16)
    )


def kernel(x: np.ndarray, f_rhs: np.ndarray) -> np.ndarray:
    assert x.shape == (B, M) and f_rhs.shape == (N,)
    wmat = _make_w(np.asarray(f_rhs, dtype=np.float32))
    xf = np.asarray(x, dtype=np.float16)
    in_maps = []
    for c in range(NCORES):
        xt = xf[c * BC : (c + 1) * BC].T  # [M, BC] grid-major
        xs = np.ascontiguousarray(
            xt.reshape(NPAIR, 2, P, BC).transpose(0, 2, 1, 3).reshape(BC, 2 * BC)
        )
        in_maps.append({"x": xs, "w": wmat})
    nc = _get_nc()
    res = bass_utils.run_bass_kernel_spmd(
        nc,
        in_maps,
        core_ids=list(range(NCORES)),
        trace=bool(int(os.environ.get("KERNEL_TRACE", "0"))),
    )
    global LAST_RESULTS
    LAST_RESULTS = res
    outs = []
    post = np.float32(H2 / SW)
    for c in range(NCORES):
        o = res.results[c]["out"]  # [BC, 2*BC] fp16, swizzled u^T
        ut = (
            np.asarray(o)
            .reshape(NPAIR, P, 2, BC)
            .transpose(0, 2, 1, 3)
            .reshape(M, BC)
        )
        outs.append(ut[:N, :].T.astype(np.float32) * post)
    return np.ascontiguousarray(np.concatenate(outs, axis=0))
